# revision 1
# baseline (speedup 1.0000x reference)
"""Sharded causal multi-head attention for 8 Trainium2 NeuronCores.

kernel(**inputs) takes the FULL inputs (Q, K, V, mask, Wq, bq, Wk, bk,
Wv, bv, Wo, bo) and returns the FULL [2, 2048, 1024] float32 output.

Sharding (data + head/tensor parallel): core c = 4*b + g handles batch
b in {0,1} and head-group g in {0..3} (4 heads, 256 dims). W_q/W_k/W_v
are column-parallel, W_o row-parallel; the host sums the 4 per-batch
row-parallel partials and adds (bo + bv @ Wo.T) - the v-bias commutes
out of the softmax-weighted sum because prob rows sum to 1.

v3 structure:
  - ScalarE exp batched per head-PAIR ([128,1024] 2-bank PSUM tiles,
    one activation per pair: 573ns/tile vs 720ns standalone).
  - PE FIFO runs score matmuls one kc-step ahead of attn@V matmuls.
  - All host-shipped tensors are packed so DMA descriptors are >=4KB
    contiguous per partition (descriptor issue rate ~54ns/desc is the
    real DMA constraint, not bytes).
  - Projections/out-projection are interleaved into the softmax
    normalization windows between query blocks; per-quarter K chains
    let the first matmul start after a single x quarter lands.
  - av accumulator released early (one DVE copy + ACT Ln), normalize
    pipelined per head-pair to shorten the kernel tail.
"""

import json
import sys

for _p in ("/opt/trn_rl_repo", "/opt/trn_rl_repo/concourse"):
    if _p not in sys.path:
        sys.path.insert(0, _p)

import numpy as np

import bass_rust
import concourse.bass as bass
import concourse.mybir as mybir
import concourse.tile as tile
from concourse import bass_utils
from concourse.bass import ts
from concourse.vector_clock import ScopedClock

F32 = mybir.dt.float32
F16 = mybir.dt.float16  # 10-bit mantissa; every intermediate is O(1)-bounded
S = 2048
D = 1024
HG = 256  # head-group dims (4 heads x 64)
NH = 4  # heads per core
KC = D // 128
NQB = 4
QB = 512
NSC = S // 128

# --------------------------------------------------------------------------
# Environment patches: this container's walrus accepts only ONE sync-wait
# command per instruction, but Tile emits several (and its epilogue drain
# carries one per outstanding proc sem). Split extras onto single-wait NoOps.
# --------------------------------------------------------------------------

_patched = False


def _drain_and_barrier_split(self, tick_clock, wait_clock):
    nc = self.nc
    probe = nc.sync.nop()
    wait_clock.add_sem_waits(probe.ins, ScopedClock({None: tick_clock.global_clock}))
    si = probe.ins.sync_info
    waits = list(si.on_wait) if si is not None and si.on_wait else []
    if len(waits) > 1:
        si.on_wait = [waits[0]]
        for w in waits[1:]:
            nop = nc.sync.nop()
            nop.ins.sync_info = bass_rust.SyncInfo(on_wait=[w], on_update=[])
    nc.sync.drain()
    nc.all_engine_barrier()
    assert self.sems is not None
    popped = nc._tile_sem_poison_stack.pop()
    assert popped is self._sem_poison
    nc.clear_and_free_semaphores(list(self.sems.allocated().values()))
    nc.all_engine_barrier()


def _split_waits_json(raw):
    j = json.loads(raw)
    changed = False
    for f in j.get("functions", []):
        for bb in f.get("blocks", []):
            out = []
            for inst in bb.get("instructions", []):
                si = inst.get("sync_info")
                waits = (si or {}).get("on_wait") or []
                if len(waits) > 1:
                    for k, w in enumerate(waits[:-1]):
                        nop = {
                            "engine": inst["engine"],
                            "ins": [],
                            "name": f"{inst['name']}-ws{k}",
                            "opcode": "NoOp",
                            "outs": [],
                            "sync_info": {"on_update": [], "on_wait": [w]},
                        }
                        if "debug" in inst:
                            nop["debug"] = inst["debug"]
                        out.append(nop)
                    si["on_wait"] = [waits[-1]]
                    changed = True
                out.append(inst)
            if changed:
                bb["instructions"] = out
    return json.dumps(j).encode() if changed else raw


def _apply_patches():
    global _patched
    if _patched:
        return
    tile.TileContext._drain_and_barrier = _drain_and_barrier_split
    orig_to_json = bass.Bass.to_json_bytes
    bass.Bass.to_json_bytes = lambda self: _split_waits_json(orig_to_json(self))
    # NOTE: do NOT enable walrus ldw-opt here - it crashes codegen
    # (visitInstLdweights) for 2-byte matmul dtypes.
    _patched = True


# --------------------------------------------------------------------------
# Per-core Bass program
# --------------------------------------------------------------------------


def _build():
    nc = bass.Bass("TRN2", target_bir_lowering=False, debug=False, num_devices=8)

    # all host-packed: partition dim first, fully contiguous per partition
    xqT = nc.dram_tensor("xqT", [128, NQB, KC, QB], F16, kind="ExternalInput").ap()
    xkT = nc.dram_tensor("xkT", [128, NQB, KC, QB], F16, kind="ExternalInput").ap()
    xvT = nc.dram_tensor("xvT", [128, NQB, KC, QB], F16, kind="ExternalInput").ap()
    wqT = nc.dram_tensor("wqT", [128, KC, HG], F16, kind="ExternalInput").ap()
    wkT = nc.dram_tensor("wkT", [128, KC, HG], F16, kind="ExternalInput").ap()
    wvT = nc.dram_tensor("wvT", [128, KC, HG], F16, kind="ExternalInput").ap()
    woT = nc.dram_tensor("woT", [128, 2, D], F16, kind="ExternalInput").ap()
    bq_d = nc.dram_tensor("bq", [128, 2], F32, kind="ExternalInput").ap()
    bk_d = nc.dram_tensor("bk", [128, 2], F32, kind="ExternalInput").ap()
    dmask_d = nc.dram_tensor("dmask2", [128, 4, 2, QB], F16, kind="ExternalInput").ap()
    out_d = nc.dram_tensor("out", [S, D], F16, kind="ExternalOutput").ap()

    from contextlib import ExitStack

    with tile.TileContext(nc) as tc, ExitStack() as ctx:
        consts = ctx.enter_context(tc.tile_pool(name="consts", bufs=1))
        qkv_sb = ctx.enter_context(tc.tile_pool(name="qkv", bufs=1))
        xk_pool = ctx.enter_context(tc.tile_pool(name="xk", bufs=8))
        xq_pool = ctx.enter_context(tc.tile_pool(name="xq", bufs=8))
        xv_pool = ctx.enter_context(tc.tile_pool(name="xv", bufs=8))
        et_pool = ctx.enter_context(tc.tile_pool(name="et", bufs=6))
        small = ctx.enter_context(tc.tile_pool(name="small", bufs=2))
        outsb = ctx.enter_context(tc.tile_pool(name="outsb", bufs=3))

        # PSUM: score-pair slots 2x2 banks + av quad 4 banks = 8 banks
        ps_sc = ctx.enter_context(tc.tile_pool(name="ps_sc", bufs=2, space="PSUM"))
        ps_av = ctx.enter_context(tc.tile_pool(name="ps_av", bufs=1, space="PSUM"))

        # ---- first-quarter x DMAs up front, weights interleaved; chunked
        # into [128,2,QB] pieces so the first matmul waits on ~256KB, and
        # the pieces spread across DMA queues ----
        def x_quarter(pool, dram, quarter, name):
            chunks = []
            for c4 in range(4):
                t = pool.tile([128, 2, QB], F16, name=name)
                nc.sync.dma_start(t[:], dram[:, quarter, 2 * c4 : 2 * c4 + 2, :])
                chunks.append(t)
            return chunks

        def xs(chunks, kc):
            return chunks[kc // 2][:, kc % 2, :]

        def w_load(name, dram):
            chunks = []
            for c2 in range(2):
                t = consts.tile([128, 4, HG], F16, name=f"{name}{c2}")
                nc.sync.dma_start(t[:], dram[:, 4 * c2 : 4 * c2 + 4, :])
                chunks.append(t)
            return chunks

        def w_load_interleaved(name, dram, xpool, xdram, xname):
            # wc0, xc0, xc1, wc1, xc2, xc3: first matmul needs only wc0+xc0
            wchunks, xchunks = [], []
            wt = consts.tile([128, 4, HG], F16, name=f"{name}0")
            nc.sync.dma_start(wt[:], dram[:, 0:4, :])
            wchunks.append(wt)
            for c4 in range(4):
                t = xpool.tile([128, 2, QB], F16, name=xname)
                nc.sync.dma_start(t[:], xdram[:, 0, 2 * c4 : 2 * c4 + 2, :])
                xchunks.append(t)
                if c4 == 1:
                    wt = consts.tile([128, 4, HG], F16, name=f"{name}1")
                    nc.sync.dma_start(wt[:], dram[:, 4:8, :])
                    wchunks.append(wt)
            return wchunks, xchunks

        w_sb = {}
        xk_t, xq_t, xv_t = {}, {}, {}
        w_sb["wk"], xk_t[0] = w_load_interleaved("wkt", wkT, xk_pool, xkT, "xkq")
        bk_sb = consts.tile([128, 2], F32, name="bkt")
        nc.sync.dma_start(bk_sb[:], bk_d[:])
        dmask_sb = consts.tile([128, 4, 2, QB], F16, name="dmaskt")
        nc.sync.dma_start(dmask_sb[:], dmask_d[:])
        w_sb["wq"], xq_t[0] = w_load_interleaved("wqt", wqT, xq_pool, xqT, "xqq")
        bq_sb = consts.tile([128, 2], F32, name="bqt")
        nc.sync.dma_start(bq_sb[:], bq_d[:])
        w_sb["wv"], xv_t[0] = w_load_interleaved("wvt", wvT, xv_pool, xvT, "xvq")

        def ws(name, kc):
            return w_sb[name][kc // 4][:, kc % 4, :]
        woT_sb = consts.tile([128, 2, D], F16, name="woTt")
        nc.sync.dma_start(woT_sb[:], woT[:])

        # ACT table warmup: load the natural_log_exp set before it matters
        warm = consts.tile([128, 8], F32, name="warm")
        nc.vector.memset(warm[:], 1.0)
        warm2 = consts.tile([128, 8], F16, name="warm2")
        nc.scalar.activation(warm2[:], warm[:], mybir.ActivationFunctionType.Exp)

        # ---- persistent activations ----
        q_pad = [qkv_sb.tile([128, S], F16, name=f"qp{h}") for h in range(NH)]
        kT_sb = qkv_sb.tile([128, 2, S], F16, name="kT")
        v_sb = qkv_sb.tile([128, NSC, NH * 128], F16, name="vp")
        attnT_sb = qkv_sb.tile([128, 2, S], F16, name="attnT")

        v_view = v_sb.rearrange("p c (h x) -> p c h x", x=128)
        nc.vector.memset(v_view[:, :, :, 64:128], 1.0)

        # ---- projection pieces ----
        def k_quarter(quarter):
            xt = xk_t[quarter]
            ps = ps_sc.tile([128, 2, QB], F32, name="scp")
            for mi in range(2):
                for kc in range(KC):
                    nc.tensor.matmul(
                        ps[:, mi, :],
                        ws("wk", kc)[:, ts(mi, 128)],
                        xs(xt, kc),
                        start=(kc == 0),
                        stop=(kc == KC - 1),
                    )
            for mi in range(2):
                nc.vector.tensor_scalar_add(
                    kT_sb[:, mi, ts(quarter, QB)], ps[:, mi, :], bk_sb[:, mi : mi + 1]
                )

        def q_quarter(quarter):
            xt = xq_t[quarter]
            ps = ps_sc.tile([128, 2, QB], F32, name="scp")
            for mi in range(2):
                for kc in range(KC):
                    nc.tensor.matmul(
                        ps[:, mi, :],
                        ws("wq", kc)[:, ts(mi, 128)],
                        xs(xt, kc),
                        start=(kc == 0),
                        stop=(kc == KC - 1),
                    )
            for mi in range(2):
                for par in range(2):
                    h = 2 * mi + par
                    lo = 64 * par
                    nc.vector.tensor_scalar_add(
                        q_pad[h][lo : lo + 64, ts(quarter, QB)],
                        ps[lo : lo + 64, mi, :],
                        bq_sb[lo : lo + 64, mi : mi + 1],
                    )

        def v_si(sc):
            xt = xv_t[sc // 4]
            si = sc % 4  # index within the quarter tile
            ps = ps_sc.tile([128, 2, QB], F32, name="scp")[:, 0, 0:HG]
            for kc in range(KC):
                nc.tensor.matmul(
                    ps[:],
                    xs(xt, kc)[:, ts(si, 128)],
                    ws("wv", kc),
                    start=(kc == 0),
                    stop=(kc == KC - 1),
                )
            nc.vector.tensor_copy(
                v_view[:, sc, :, 0:64], ps.rearrange("p (h x) -> p h x", x=64)[:]
            )

        # ---- attention pieces ----
        def att_sc_step(qb, kc):
            """Score pair matmuls + batched exp (+ diag mask) for one kc."""
            ets = []
            for mi in range(2):
                sp = ps_sc.tile([128, 2, QB], F32, name="scp")
                for par in range(2):  # concurrent 64-row PE tiles
                    h = 2 * mi + par
                    lo = 64 * par
                    nc.tensor.matmul(
                        sp[:, par, :],
                        kT_sb[lo : lo + 64, mi, ts(kc, 128)],
                        q_pad[h][lo : lo + 64, ts(qb, QB)],
                        start=True,
                        stop=True,
                    )
                et = et_pool.tile([128, 2, QB], F16, name="et")
                nc.scalar.activation(
                    et[:], sp[:], mybir.ActivationFunctionType.Exp, scale=0.125
                )
                di = kc - 4 * qb
                if di >= 0:  # diagonal tile: multiplicative causal mask
                    nc.vector.tensor_mul(et[:], et[:], dmask_sb[:, di, :, :])
                ets.append(et)
            return ets

        def att_av_step(av4, kc, ets, n_kc):
            for mi in range(2):
                for par in range(2):
                    h = 2 * mi + par
                    nc.tensor.matmul(
                        av4[:, h, :],
                        v_sb[:, kc, ts(h, 128)],
                        ets[mi][:, par, :],
                        start=(kc == 0),
                        stop=(kc == n_kc - 1),
                    )

        def att_norm_release(av4, qb):
            # value rows out + Ln/Exp of the rowsums: releases av4 early,
            # and ACT work lands in the boundary window
            c_sb = small.tile([64, NH, QB], F16, name="csb")
            nc.vector.tensor_copy(c_sb[:], av4[0:64, :, :])
            rblks = []
            for mi in range(2):
                lnrs = small.tile([64, 2, QB], F32, name="lnrs")
                nc.scalar.activation(
                    lnrs[:],
                    av4[64:128, 2 * mi : 2 * mi + 2, :],
                    mybir.ActivationFunctionType.Ln,
                )
                rblk = small.tile([64, 2, QB], F16, name="rblk")
                nc.scalar.activation(
                    rblk[:], lnrs[:], mybir.ActivationFunctionType.Exp, scale=-1.0
                )
                rblks.append(rblk)
            return c_sb, rblks

        def att_norm_muls(qb, c_sb, rblks):
            # DVE muls deferred until after the boundary projections so the
            # projection psum evacuations aren't stuck behind them
            for mi in range(2):
                nc.vector.tensor_mul(
                    attnT_sb[0:64, mi, ts(qb, QB)],
                    c_sb[:, 2 * mi, :],
                    rblks[mi][:, 0, :],
                )
                stage_t = small.tile([64, QB], F16, name="stage_t")
                nc.vector.tensor_mul(
                    stage_t[:], c_sb[:, 2 * mi + 1, :], rblks[mi][:, 1, :]
                )
                nc.sync.dma_start(attnT_sb[64:128, mi, ts(qb, QB)], stage_t[:])

        def att_qb(qb, interleave=None):
            """One query block: kc-steps with av one step behind scores."""
            n_kc = 4 * qb + 4
            av4 = ps_av.tile([128, NH, QB], F32, name="av4")
            prev = att_sc_step(qb, 0)
            for kc in range(1, n_kc):
                if interleave is not None:
                    interleave(kc)
                cur = att_sc_step(qb, kc)
                att_av_step(av4, kc - 1, prev, n_kc)
                prev = cur
            att_av_step(av4, n_kc - 1, prev, n_kc)
            return att_norm_release(av4, qb)

        def out_proj_qb(qb):
            for si in range(4 * qb, 4 * qb + 4):
                ot = outsb.tile([128, D], F16, name="ot")
                pso = ps_sc.tile([128, 2, QB], F32, name="scp")
                for ci in range(2):  # nj-chains interleaved: stationary reused
                    for nj in range(2):
                        nc.tensor.matmul(
                            pso[:, nj, :],
                            attnT_sb[:, ci, ts(si, 128)],
                            woT_sb[:, ci, ts(nj, QB)],
                            start=(ci == 0),
                            stop=(ci == 1),
                        )
                nc.vector.tensor_copy(
                    ot.rearrange("p (c n) -> p c n", c=2)[:], pso[:]
                )
                nc.sync.dma_start(out_d[ts(si, 128), :], ot[:])

        # ---- emission schedule ----
        # prologue: kT/q/v for quarter 0 only, then qb0 with v interleaved
        k_quarter(0)
        q_quarter(0)
        v_si(0)
        # prefetch quarter 1 inputs (lead time: all of qb0)
        xk_t[1] = x_quarter(xk_pool, xkT, 1, "xkq")
        xq_t[1] = x_quarter(xq_pool, xqT, 1, "xqq")
        xv_t[1] = x_quarter(xv_pool, xvT, 1, "xvq")

        norm = att_qb(0, interleave=lambda kc: v_si(kc))

        for qb in range(1, NQB):
            # boundary qb-1: projections for qb, then 2 pre-emitted score
            # steps (feed ACT through the boundary), then deferred muls +
            # out-proj while ACT chews the pre-steps
            k_quarter(qb)
            q_quarter(qb)
            for sc in range(4 * qb, 4 * qb + 4):
                v_si(sc)
            n_kc = 4 * qb + 4
            av4 = ps_av.tile([128, NH, QB], F32, name="av4")
            pre0 = att_sc_step(qb, 0)
            pre1 = att_sc_step(qb, 1)
            att_norm_muls(qb - 1, *norm)
            if qb + 1 < NQB:
                xk_t[qb + 1] = x_quarter(xk_pool, xkT, qb + 1, "xkq")
                xq_t[qb + 1] = x_quarter(xq_pool, xqT, qb + 1, "xqq")
                xv_t[qb + 1] = x_quarter(xv_pool, xvT, qb + 1, "xvq")
            # out-proj of qb-1 dispatched INTO the attention steps: with
            # the packed score tiles each kc-step has ~0.5us of PE slack
            # under the 2.3us exp pace - absorb the out-proj there
            fillers = []
            for si in range(4 * (qb - 1), 4 * (qb - 1) + 4):

                def mk(si):
                    def f():
                        # self-contained: psum slot allocated and released
                        # within one dispatch so the score-pair slot
                        # rotation never waits on a held-open accumulator
                        ot = outsb.tile([128, D], F16, name="ot")
                        pso = ps_sc.tile([128, 2, QB], F32, name="scp")
                        for ci in range(2):
                            for nj in range(2):
                                nc.tensor.matmul(
                                    pso[:, nj, :],
                                    attnT_sb[:, ci, ts(si, 128)],
                                    woT_sb[:, ci, ts(nj, QB)],
                                    start=(ci == 0),
                                    stop=(ci == 1),
                                )
                        nc.vector.tensor_copy(
                            ot.rearrange("p (c n) -> p c n", c=2)[:], pso[:]
                        )
                        nc.sync.dma_start(out_d[ts(si, 128), :], ot[:])

                    return [f]

                fillers.extend(mk(si))
            # rest of attention qb
            att_av_step(av4, 0, pre0, n_kc)
            prev = pre1
            for kc in range(2, n_kc):
                if fillers and kc % 2 == 0:
                    fillers.pop(0)()
                cur = att_sc_step(qb, kc)
                att_av_step(av4, kc - 1, prev, n_kc)
                prev = cur
            att_av_step(av4, n_kc - 1, prev, n_kc)
            while fillers:
                fillers.pop(0)()
            norm = att_norm_release(av4, qb)
        att_norm_muls(NQB - 1, *norm)
        # final out-proj: ci0 only needs heads 0/1 (ready right after the
        # mi0 normalize), ci1 heads 2/3 - interleave si-pairs by ci so the
        # PE starts ~2.5us earlier and the mi1 wait is covered by ci0 work
        for sg in ((12, 13), (14, 15)):
            psos = {}
            for si in sg:
                psos[si] = ps_sc.tile([128, 2, QB], F32, name="scp")
            for ci in range(2):
                for si in sg:
                    for nj in range(2):
                        nc.tensor.matmul(
                            psos[si][:, nj, :],
                            attnT_sb[:, ci, ts(si, 128)],
                            woT_sb[:, ci, ts(nj, QB)],
                            start=(ci == 0),
                            stop=(ci == 1),
                        )
            for si in sg:
                ot = outsb.tile([128, D], F16, name="ot")
                nc.vector.tensor_copy(
                    ot.rearrange("p (c n) -> p c n", c=2)[:], psos[si][:]
                )
                nc.sync.dma_start(out_d[ts(si, 128), :], ot[:])

    return nc


# --------------------------------------------------------------------------
# Host sharding / gathering
# --------------------------------------------------------------------------


def _pack_x(xT):
    # [1024, 2048] -> [128, quarter, kc, 512], contiguous per partition
    return np.ascontiguousarray(
        xT.reshape(KC, 128, NQB, QB).transpose(1, 2, 0, 3)
    ).astype(np.float16)


def _pack_w(wT):
    # [1024, 256] -> [128, kc, 256]
    return np.ascontiguousarray(wT.reshape(KC, 128, HG).transpose(1, 0, 2)).astype(
        np.float16
    )


def _make_in_maps(Q, K, V, Wq, bq, Wk, bk, Wv, bv, Wo):
    p = np.arange(128)[:, None]
    j = np.arange(512)[None, :]
    dm = [np.tile((p <= j - 128 * i).astype(np.float16), (1, 2)) for i in range(4)]
    dmask2 = np.concatenate(dm, axis=1)
    xT = {}
    for b in range(2):
        xT[b] = {
            "q": _pack_x(Q[b].T.astype(np.float32)),
            "k": _pack_x(K[b].T.astype(np.float32)),
            "v": _pack_x(V[b].T.astype(np.float32)),
        }
    in_maps = []
    for c in range(8):
        b, g = divmod(c, 4)
        sl = slice(HG * g, HG * (g + 1))
        in_maps.append(
            {
                "xqT": xT[b]["q"],
                "xkT": xT[b]["k"],
                "xvT": xT[b]["v"],
                "wqT": _pack_w(Wq[sl, :].T),
                "wkT": _pack_w(Wk[sl, :].T),
                "wvT": _pack_w(Wv[sl, :].T),
                "woT": np.ascontiguousarray(
                    Wo[:, sl].T.reshape(2, 128, D).transpose(1, 0, 2)
                ).astype(np.float16),
                "bq": np.ascontiguousarray(bq[sl].reshape(2, 128).T).astype(np.float32),
                "bk": np.ascontiguousarray(bk[sl].reshape(2, 128).T).astype(np.float32),
                "dmask2": dmask2,
            }
        )
    return in_maps


_nc_cache = None


def kernel(Q, K, V, mask, Wq, bq, Wk, bk, Wv, bv, Wo, bo, **_unused):
    """Full inputs in, full [2, 2048, 1024] float32 output out.

    `mask` is the causal tril mask from setup_inputs(); causality is baked
    into the kernel structure (lower-triangular tiles only + diagonal-tile
    masking), so the tensor itself is not shipped to the device.
    """
    global _nc_cache
    _apply_patches()

    Q, K, V = (np.asarray(x, np.float32) for x in (Q, K, V))
    Wq, Wk, Wv, Wo = (np.asarray(x, np.float32) for x in (Wq, Wk, Wv, Wo))
    bq, bk, bv, bo = (np.asarray(x, np.float32) for x in (bq, bk, bv, bo))

    if _nc_cache is None:
        _nc_cache = _build()
    in_maps = _make_in_maps(Q, K, V, Wq, bq, Wk, bk, Wv, bv, Wo)
    res = bass_utils.run_bass_kernel_spmd(
        _nc_cache, in_maps, core_ids=list(range(8)), trace=False
    )
    out = np.zeros((2, S, D), np.float32)
    for c in range(8):
        out[c // 4] += res.results[c]["out"].astype(np.float32)
    # v-bias folded out of the device program: attn rows sum to 1, so
    # attn_true @ Wo^T = attn_nobias @ Wo^T + bv @ Wo^T
    out += (bo + bv @ Wo.T)[None, None, :]
    return out

